# revision 4
# baseline (speedup 1.0000x reference)
"""Trainium2 Bass kernel for the Bolt 64-QAM demapper MLP forward pass.

Problem: llr = (relu(relu(z @ W1 + b1) @ W2 + b2) @ W3 + b3).reshape(B, S*6)
  z [4096, 512, 3] f32, W1 [3,128], W2 [128,128], W3 [128,6].

Strategy: pure data parallel over 8 NeuronCores (batch split), each core
processes 262144 rows through a feature-major PE pipeline:

Per tile t (2048 rows), row R = t*2048 + p*16 + beta (p<128 partitions):
  1. contiguous DMA load z_raw [128, 48]  (16 rows * 3 coords per partition)
  2. GPSIMD expand -> z_in[p, 32*beta + u] = z_raw[p, 12*beta + u], u<3
  3. DVE 32x32 block transpose -> zTT[32a + u, 32*beta + v]  (coords now on
     partitions at quarter-aligned bases; a = p>>5, v = p&31)
  4. L1: 4 row-packed K=3 fp32r matmuls (tile_position (32a,0)), N=512
     -> h1 PSUM [128,1024] (a-pairs); ACT/DVE evac fused relu+bias
  5. L2: K=128 fp32r matmuls N=512 -> h2 PSUM; fused relu+bias evac
  6. L3: 4 col-packed K=128 M=32 (W3 zero-padded) matmuls -> strips 32a of
     one PSUM bank; DVE evac fused +b3; DVE block transpose back to
     row-major; single contiguous-run DMA store (96 f32 per partition run)
"""
import os
import numpy as np
from contextlib import ExitStack

import concourse.bacc as bacc
import concourse.mybir as mybir
import concourse.tile as tile
from concourse import bass_utils

F32 = mybir.dt.float32
F32R = mybir.dt.float32r
BF16 = mybir.dt.bfloat16
AF = mybir.ActivationFunctionType
ALU = mybir.AluOpType

N_CORES = 8
B, S, H, NB = 4096, 512, 128, 6
ROWS_TOTAL = B * S                    # 2097152
ROWS_CORE = ROWS_TOTAL // N_CORES     # 262144
TROWS = 2048                          # rows per tile
NT = ROWS_CORE // TROWS               # 128 tiles

LAST_RESULTS = None  # stashed BassKernelResults for test harness inspection


def _build_nc():
    nc = bacc.Bacc("TRN2", target_bir_lowering=False, debug=False, num_devices=N_CORES)
    z_d = nc.dram_tensor("z", [ROWS_CORE, 3], F32, kind="ExternalInput")
    w1rep_d = nc.dram_tensor("w1rep", [128, H], BF16, kind="ExternalInput")
    b1_d = nc.dram_tensor("b1", [H, 1], F32, kind="ExternalInput")
    w2_d = nc.dram_tensor("w2", [H, H], BF16, kind="ExternalInput")
    b2_d = nc.dram_tensor("b2", [H, 1], F32, kind="ExternalInput")
    w3_d = nc.dram_tensor("w3", [H, 32], BF16, kind="ExternalInput")
    b3rep_d = nc.dram_tensor("b3rep", [128, 1], F32, kind="ExternalInput")
    out_d = nc.dram_tensor("out", [ROWS_CORE, NB], F32, kind="ExternalOutput")

    with tile.TileContext(nc) as tc, ExitStack() as ctx:
        const = ctx.enter_context(tc.tile_pool(name="const", bufs=1))
        zp = ctx.enter_context(tc.tile_pool(name="zp", bufs=3))
        hp = ctx.enter_context(tc.tile_pool(name="hp", bufs=3))
        op = ctx.enter_context(tc.tile_pool(name="op", bufs=3))
        ps_h1 = ctx.enter_context(tc.tile_pool(name="ps_h1", bufs=2, space="PSUM"))
        ps_h2 = ctx.enter_context(tc.tile_pool(name="ps_h2", bufs=1, space="PSUM"))
        ps_o = ctx.enter_context(tc.tile_pool(name="ps_o", bufs=2, space="PSUM"))

        w1rep = const.tile([128, H], BF16)
        nc.sync.dma_start(w1rep[:], w1rep_d.ap())
        w2sb = const.tile([H, H], BF16)
        nc.sync.dma_start(w2sb[:], w2_d.ap())
        w3sb = const.tile([H, 32], BF16)
        nc.sync.dma_start(w3sb[:], w3_d.ap())
        b1sb = const.tile([H, 1], F32)
        nc.sync.dma_start(b1sb[:], b1_d.ap())
        b2sb = const.tile([H, 1], F32)
        nc.sync.dma_start(b2sb[:], b2_d.ap())
        b3rep = const.tile([128, 1], F32)
        nc.sync.dma_start(b3rep[:], b3rep_d.ap())

        z_flat = z_d.ap().rearrange("(t p r) i -> t p (r i)", t=NT, p=128)
        out_v = out_d.ap().rearrange("(t p r) o -> t p (r o)", t=NT, p=128)

        for t in range(NT):
            z_raw = zp.tile([128, 48], F32, tag="zraw")
            nc.sync.dma_start(z_raw[:], z_flat[t])
            z_in = zp.tile([128, 512], BF16, tag="zin")
            nc.gpsimd.memset(z_in[:], 0.0)
            z_in_v = z_in[:].rearrange("p (r u) -> p r u", r=16)[:, :, 0:3]
            z_raw_v = z_raw[:].rearrange("p (r i) -> p r i", r=16)
            nc.gpsimd.tensor_copy(z_in_v, z_raw_v)
            zTT = zp.tile([128, 512], BF16, tag="zTT")
            nc.vector.transpose(zTT[:], z_in[:])

            # L1: a = 2P + s
            h1_sbs = []
            for P in range(2):
                h1_ps = ps_h1.tile([128, 1024], F32)
                for s in range(2):
                    a = 2 * P + s
                    nc.tensor.matmul(
                        h1_ps[:, s * 512 : (s + 1) * 512],
                        w1rep[32 * a : 32 * a + 3, :],
                        zTT[32 * a : 32 * a + 3, :],
                        tile_position=(32 * a, 0),
                    )
                h1_sb = hp.tile([128, 1024], BF16, tag="h1")
                nc.scalar.activation(h1_sb[:], h1_ps[:], AF.Relu, bias=b1sb[:])
                h1_sbs.append(h1_sb)

            # L2 (h2 evac split between ACT and DVE for engine balance)
            h2_sbs = []
            for P in range(2):
                h2_ps = ps_h2.tile([128, 1024], F32)
                for s in range(2):
                    nc.tensor.matmul(
                        h2_ps[:, s * 512 : (s + 1) * 512],
                        w2sb[:],
                        h1_sbs[P][:, s * 512 : (s + 1) * 512],
                    )
                h2_sb = hp.tile([128, 1024], BF16, tag="h2")
                if P == 0:
                    nc.scalar.activation(h2_sb[:], h2_ps[:], AF.Relu, bias=b2sb[:])
                else:
                    nc.vector.tensor_scalar(
                        h2_sb[:], h2_ps[:], b2sb[:], 0.0, op0=ALU.add, op1=ALU.max
                    )
                h2_sbs.append(h2_sb)

            # L3: strips a (W3 zero-padded to M=32 so the full bank is written)
            out_ps = ps_o.tile([128, 512], F32)
            for P in range(2):
                for s in range(2):
                    a = 2 * P + s
                    nc.tensor.matmul(
                        out_ps[32 * a : 32 * a + 32, :],
                        w3sb[:],
                        h2_sbs[P][:, s * 512 : (s + 1) * 512],
                        tile_position=(0, 32 * a),
                    )
            out_sb = op.tile([128, 512], F32, tag="osb")
            nc.vector.tensor_scalar(out_sb[:], out_ps[:], b3rep[:], None, op0=ALU.add)
            outT = op.tile([128, 512], F32, tag="oT")
            nc.vector.transpose(outT[:], out_sb[:])
            src_o = outT[:].rearrange("p (r u) -> p r u", r=16)[:, :, 0:NB]
            nc.sync.dma_start(out_v[t].rearrange("p (r o) -> p r o", r=16), src_o)

    nc.compile()
    return nc


def kernel(z, W1, b1, W2, b2, W3, b3):
    global LAST_RESULTS
    z = np.asarray(z, dtype=np.float32)
    W1 = np.asarray(W1, dtype=np.float32)
    b1 = np.asarray(b1, dtype=np.float32)
    W2 = np.asarray(W2, dtype=np.float32)
    b2 = np.asarray(b2, dtype=np.float32)
    W3 = np.asarray(W3, dtype=np.float32)
    b3 = np.asarray(b3, dtype=np.float32)

    # host-side weight prep (tiny)
    w1rep = np.zeros((128, H), mybir.dt.np(BF16))
    for a in range(4):
        w1rep[32 * a : 32 * a + 3] = W1.astype(mybir.dt.np(BF16))
    w3pad = np.zeros((H, 32), mybir.dt.np(BF16))
    w3pad[:, :NB] = W3.astype(mybir.dt.np(BF16))
    b3rep = np.zeros((128, 1), np.float32)
    for a in range(4):
        b3rep[32 * a : 32 * a + NB, 0] = b3

    z_rows = np.ascontiguousarray(z).reshape(ROWS_TOTAL, 3)
    shards = np.split(z_rows, N_CORES, axis=0)

    common = {
        "w1rep": w1rep,
        "b1": np.ascontiguousarray(b1.reshape(H, 1)),
        "w2": np.ascontiguousarray(W2.astype(mybir.dt.np(BF16))),
        "b2": np.ascontiguousarray(b2.reshape(H, 1)),
        "w3": w3pad,
        "b3rep": b3rep,
    }
    in_maps = [dict(common, z=np.ascontiguousarray(s)) for s in shards]

    nc = _build_nc()
    res = bass_utils.run_bass_kernel_spmd(
        nc,
        in_maps,
        core_ids=list(range(N_CORES)),
        trace=bool(os.environ.get("KBENCH_TRACE")),
    )
    LAST_RESULTS = res
    outs = [res.results[i]["out"] for i in range(N_CORES)]
    full = np.concatenate(outs, axis=0)  # [ROWS_TOTAL, 6]
    return full.reshape(B, S * NB).astype(np.float32)


# revision 5
# speedup vs baseline: 1.4532x; 1.4532x over previous
"""Trainium2 Bass kernel for the Bolt 64-QAM demapper MLP forward pass.

Problem: llr = (relu(relu(z @ W1 + b1) @ W2 + b2) @ W3 + b3).reshape(B, S*6)
  z [4096, 512, 3] f32, W1 [3,128], W2 [128,128], W3 [128,6].

Strategy: pure data parallel over 8 NeuronCores (batch split), each core
processes 262144 rows through a feature-major PE pipeline:

Per tile t (2048 rows), row R = t*2048 + p*16 + beta (p<128 partitions):
  1. contiguous DMA load z_raw [128, 48]  (16 rows * 3 coords per partition)
  2. GPSIMD expand -> z_in[p, 32*beta + u] = z_raw[p, 12*beta + u], u<3
  3. DVE 32x32 block transpose -> zTT[32a + u, 32*beta + v]  (coords now on
     partitions at quarter-aligned bases; a = p>>5, v = p&31)
  4. L1: 4 row-packed K=3 fp32r matmuls (tile_position (32a,0)), N=512
     -> h1 PSUM [128,1024] (a-pairs); ACT/DVE evac fused relu+bias
  5. L2: K=128 fp32r matmuls N=512 -> h2 PSUM; fused relu+bias evac
  6. L3: 4 col-packed K=128 M=32 (W3 zero-padded) matmuls -> strips 32a of
     one PSUM bank; DVE evac fused +b3; DVE block transpose back to
     row-major; single contiguous-run DMA store (96 f32 per partition run)
"""
import os
import numpy as np
from contextlib import ExitStack

import concourse.bacc as bacc
import concourse.mybir as mybir
import concourse.tile as tile
from concourse import bass_utils

F32 = mybir.dt.float32
F32R = mybir.dt.float32r
BF16 = mybir.dt.bfloat16
AF = mybir.ActivationFunctionType
ALU = mybir.AluOpType

N_CORES = 8
B, S, H, NB = 4096, 512, 128, 6
ROWS_TOTAL = B * S                    # 2097152
ROWS_CORE = ROWS_TOTAL // N_CORES     # 262144
TROWS = 2048                          # rows per tile
NT = ROWS_CORE // TROWS               # 128 tiles

LAST_RESULTS = None  # stashed BassKernelResults for test harness inspection


def _build_nc():
    nc = bacc.Bacc("TRN2", target_bir_lowering=False, debug=False, num_devices=N_CORES)
    z_d = nc.dram_tensor("z", [ROWS_CORE, 3], F32, kind="ExternalInput")
    w1rep_d = nc.dram_tensor("w1rep", [128, H], BF16, kind="ExternalInput")
    b1_d = nc.dram_tensor("b1", [H, 1], F32, kind="ExternalInput")
    w2_d = nc.dram_tensor("w2", [H, H], BF16, kind="ExternalInput")
    b2_d = nc.dram_tensor("b2", [H, 1], F32, kind="ExternalInput")
    w3_d = nc.dram_tensor("w3", [H, 32], BF16, kind="ExternalInput")
    b3tile_d = nc.dram_tensor("b3tile", [128, 96], F32, kind="ExternalInput")
    out_d = nc.dram_tensor("out", [ROWS_CORE, NB], F32, kind="ExternalOutput")

    with tile.TileContext(nc) as tc, ExitStack() as ctx:
        const = ctx.enter_context(tc.tile_pool(name="const", bufs=1))
        zp = ctx.enter_context(tc.tile_pool(name="zp", bufs=3))
        hp = ctx.enter_context(tc.tile_pool(name="hp", bufs=3))
        op = ctx.enter_context(tc.tile_pool(name="op", bufs=3))
        ps_h1 = ctx.enter_context(tc.tile_pool(name="ps_h1", bufs=2, space="PSUM"))
        ps_h2 = ctx.enter_context(tc.tile_pool(name="ps_h2", bufs=1, space="PSUM"))
        ps_o = ctx.enter_context(tc.tile_pool(name="ps_o", bufs=2, space="PSUM"))

        w1rep = const.tile([128, H], BF16)
        nc.sync.dma_start(w1rep[:], w1rep_d.ap())
        w2sb = const.tile([H, H], BF16)
        nc.sync.dma_start(w2sb[:], w2_d.ap())
        w3sb = const.tile([H, 32], BF16)
        nc.sync.dma_start(w3sb[:], w3_d.ap())
        b1sb = const.tile([H, 1], F32)
        nc.sync.dma_start(b1sb[:], b1_d.ap())
        b2sb = const.tile([H, 1], F32)
        nc.sync.dma_start(b2sb[:], b2_d.ap())
        b3tile = const.tile([128, 96], F32)
        nc.sync.dma_start(b3tile[:], b3tile_d.ap())

        NQ = NT // 4
        z_flat = z_d.ap().rearrange("(q p m) i -> q p (m i)", q=NQ, p=128)
        out_v = out_d.ap().rearrange("(q p m) o -> q p (m o)", q=NQ, p=128)

        for q in range(NT // 4):
          z_raw = zp.tile([128, 192], F32, tag="zraw")
          nc.sync.dma_start(z_raw[:], z_flat[q])
          outc = op.tile([128, 384], F32, tag="outc")
          for j in range(4):
            t = 4 * q + j
            z_in = zp.tile([128, 512], BF16, tag="zin")
            nc.gpsimd.memset(z_in[:], 0.0)
            z_in_v = z_in[:].rearrange("p (r u) -> p r u", r=16)[:, :, 0:3]
            z_raw_v = z_raw[:].rearrange("p (m i) -> p m i", m=64)[:, 16 * j : 16 * (j + 1), :]
            nc.gpsimd.tensor_copy(z_in_v, z_raw_v)
            zTT = zp.tile([128, 512], BF16, tag="zTT")
            nc.vector.transpose(zTT[:], z_in[:])

            # L1: a = 2P + s
            h1_sbs = []
            for P in range(2):
                h1_ps = ps_h1.tile([128, 1024], F32)
                for s in range(2):
                    a = 2 * P + s
                    nc.tensor.matmul(
                        h1_ps[:, s * 512 : (s + 1) * 512],
                        w1rep[32 * a : 32 * a + 3, :],
                        zTT[32 * a : 32 * a + 3, :],
                        tile_position=(32 * a, 0),
                    )
                h1_sb = hp.tile([128, 1024], BF16, tag="h1")
                nc.scalar.activation(h1_sb[:], h1_ps[:], AF.Relu, bias=b1sb[:])
                h1_sbs.append(h1_sb)

            # L2 (h2 evac split between ACT and DVE for engine balance)
            h2_sbs = []
            for P in range(2):
                h2_ps = ps_h2.tile([128, 1024], F32)
                for s in range(2):
                    nc.tensor.matmul(
                        h2_ps[:, s * 512 : (s + 1) * 512],
                        w2sb[:],
                        h1_sbs[P][:, s * 512 : (s + 1) * 512],
                    )
                h2_sb = hp.tile([128, 1024], BF16, tag="h2")
                on_act = (P == 0 and t % 4 != 3)
                if on_act:
                    nc.scalar.activation(h2_sb[:], h2_ps[:], AF.Relu, bias=b2sb[:])
                else:
                    nc.vector.tensor_scalar(
                        h2_sb[:], h2_ps[:], b2sb[:], 0.0, op0=ALU.add, op1=ALU.max
                    )
                h2_sbs.append(h2_sb)

            # L3: strips a (W3 zero-padded to M=32 so the full bank is written)
            out_ps = ps_o.tile([128, 512], F32)
            for P in range(2):
                for s in range(2):
                    a = 2 * P + s
                    nc.tensor.matmul(
                        out_ps[32 * a : 32 * a + 32, :],
                        w3sb[:],
                        h2_sbs[P][:, s * 512 : (s + 1) * 512],
                        tile_position=(0, 32 * a),
                    )
            outT = op.tile([128, 512], F32, tag="oT")
            nc.vector.transpose(outT[:], out_ps[:])
            # pack + bias on gpsimd: outc[:, 96j + 6r + o] = outT[p, 32r+o] + b3
            src_pk = outT[:].rearrange("p (r u) -> p r u", r=16)[:, :, 0:NB]
            dst_pk = outc[:, 96 * j : 96 * (j + 1)].rearrange("p (r o) -> p r o", r=16)
            b3_v = b3tile[:].rearrange("p (r o) -> p r o", r=16)
            nc.gpsimd.tensor_add(dst_pk, src_pk, b3_v)
          nc.sync.dma_start(out_v[q], outc[:])

    nc.compile()
    return nc


def kernel(z, W1, b1, W2, b2, W3, b3):
    global LAST_RESULTS
    z = np.asarray(z, dtype=np.float32)
    W1 = np.asarray(W1, dtype=np.float32)
    b1 = np.asarray(b1, dtype=np.float32)
    W2 = np.asarray(W2, dtype=np.float32)
    b2 = np.asarray(b2, dtype=np.float32)
    W3 = np.asarray(W3, dtype=np.float32)
    b3 = np.asarray(b3, dtype=np.float32)

    # host-side weight prep (tiny)
    w1rep = np.zeros((128, H), mybir.dt.np(BF16))
    for a in range(4):
        w1rep[32 * a : 32 * a + 3] = W1.astype(mybir.dt.np(BF16))
    w3pad = np.zeros((H, 32), mybir.dt.np(BF16))
    w3pad[:, :NB] = W3.astype(mybir.dt.np(BF16))
    b3tile = np.tile(b3.astype(np.float32), (128, 16))  # [128, 96]

    z_rows = np.ascontiguousarray(z).reshape(ROWS_TOTAL, 3)
    shards = np.split(z_rows, N_CORES, axis=0)

    common = {
        "w1rep": w1rep,
        "b1": np.ascontiguousarray(b1.reshape(H, 1)),
        "w2": np.ascontiguousarray(W2.astype(mybir.dt.np(BF16))),
        "b2": np.ascontiguousarray(b2.reshape(H, 1)),
        "w3": w3pad,
        "b3tile": np.ascontiguousarray(b3tile),
    }
    in_maps = [dict(common, z=np.ascontiguousarray(s)) for s in shards]

    nc = _build_nc()
    res = bass_utils.run_bass_kernel_spmd(
        nc,
        in_maps,
        core_ids=list(range(N_CORES)),
        trace=bool(os.environ.get("KBENCH_TRACE")),
    )
    LAST_RESULTS = res
    outs = [res.results[i]["out"] for i in range(N_CORES)]
    full = np.concatenate(outs, axis=0)  # [ROWS_TOTAL, 6]
    return full.reshape(B, S * NB).astype(np.float32)
